# revision 8
# baseline (speedup 1.0000x reference)
"""CentroidTripletLoss on 8 Trainium2 NeuronCores (Bass/Tile).

Data-parallel over batch B=32768 across 8 cores (4096 rows each), cov_inv
replicated. Per core: one-hot matmul segment-sums -> AllReduce (C*D + C) ->
one-hot gather matmuls with PSUM-assembled (gather - E) -> fused DVE diff
build -> PE transposes -> fp32r matmuls diff @ cov_inv -> fused multiply +
free-axis-accumulate for the Mahalanobis quadratic form -> hinge epilogue.
Each core emits its partial hinge sum; the host sums and divides by B.
"""
import sys

sys.path.insert(0, "/opt/trn_rl_repo")

import numpy as np

import concourse.bass as bass
import concourse.mybir as mybir
import concourse.tile as tile
from concourse.bass_utils import run_bass_kernel_spmd
from concourse.masks import make_identity

N_CORES = 8
B, D, C = 32768, 512, 128
R = B // N_CORES            # rows per core (4096)
P = 128                     # partitions / chunk rows
NCHUNK = R // P             # 32 chunks per core
DB = D // P                 # 4 d-blocks
MARGIN = 0.1
EPS = 1e-12

F32 = mybir.dt.float32
F32R = mybir.dt.float32r
I32 = mybir.dt.int32
AF = mybir.ActivationFunctionType
OP = mybir.AluOpType


def _patch_drain_and_barrier():
    """The walrus build in this container rejects >1 sync wait per
    instruction; Tile's kernel-tail drain piles every outstanding semaphore
    wait onto one SP Drain. Split it into a chain of single-wait drains."""
    import bass_rust as _br
    from concourse.vector_clock import ScopedClock

    def _patched(self, tick_clock, wait_clock):
        nc = self.nc
        drain_inst = nc.sync.drain()
        wait_clock.add_sem_waits(
            drain_inst.ins, ScopedClock({None: tick_clock.global_clock}))
        waits = list(drain_inst.ins.sync_info.on_wait)
        if len(waits) > 1:
            drain_inst.ins.sync_info.on_wait = waits[:1]
            for w in waits[1:]:
                d = nc.sync.drain()
                if d.ins.sync_info is None:
                    d.ins.sync_info = _br.SyncInfo(on_wait=[w], on_update=[])
                else:
                    d.ins.sync_info.on_wait = [w]
        nc.all_engine_barrier()
        popped = nc._tile_sem_poison_stack.pop()
        assert popped is self._sem_poison
        nc.clear_and_free_semaphores(list(self.sems.allocated().values()))
        nc.all_engine_barrier()

    tile.TileContext._drain_and_barrier = _patched


_patch_drain_and_barrier()


def _split_multi_waits(nc):
    """The walrus build here accepts 1 sync wait per instruction (2 for
    EventSemaphore). Move extra waits onto same-engine NoOps inserted just
    before the over-subscribed instruction."""
    import bass_rust as _br
    k = 0
    for fn in nc.m.functions:
        for bb in fn.blocks:
            insts = bb.instructions
            out = []
            changed = False
            for ins in insts:
                si = ins.sync_info
                cap = 2 if isinstance(ins, mybir.InstEventSemaphore) else 1
                if si is not None and len(si.on_wait) > cap:
                    waits = list(si.on_wait)
                    for w in waits[:-cap]:
                        nop = mybir.InstNoOp(name=f"swsplit_{k}", ins=[], outs=[])
                        k += 1
                        nop.engine = ins.engine
                        nop.sync_info = _br.SyncInfo(on_wait=[w], on_update=[])
                        out.append(nop)
                    si.on_wait = waits[-cap:]
                    changed = True
                out.append(ins)
            if changed:
                bb.instructions = out


def r32(ap):
    return ap.bitcast(F32R)


def build():
    nc = bass.Bass("TRN2", target_bir_lowering=False, debug=False,
                   num_devices=N_CORES)
    anchor = nc.dram_tensor("anchor", [R, D], F32, kind="ExternalInput").ap()
    emb = nc.dram_tensor("embeddings", [R, D], F32, kind="ExternalInput").ap()
    cov = nc.dram_tensor("cov_inv", [D, D], F32, kind="ExternalInput").ap()
    labels = nc.dram_tensor("labels", [R], I32, kind="ExternalInput").ap()
    neg = nc.dram_tensor("neg_labels", [R], I32, kind="ExternalInput").ap()
    out = nc.dram_tensor("partial", [1, 1], F32, kind="ExternalOutput").ap()

    from contextlib import ExitStack
    with tile.TileContext(nc) as tc, ExitStack() as ctx:
        cst = ctx.enter_context(tc.tile_pool(name="cst", bufs=1))
        big = ctx.enter_context(tc.tile_pool(name="big", bufs=1))
        wp = ctx.enter_context(tc.tile_pool(name="wp", bufs=3))
        dram = ctx.enter_context(tc.tile_pool(name="dram", bufs=1, space="DRAM"))

        # ---- resident loads ----
        E_sb = big.tile([P, NCHUNK, D], F32)   # embeddings, chunk-major
        A_sb = big.tile([P, NCHUNK, D], F32)   # anchors
        Q_sb = big.tile([P, DB, D], F32)       # cov_inv row-blocks
        emb_r = emb.rearrange("(c p) d -> p c d", p=P)
        anch_r = anchor.rearrange("(c p) d -> p c d", p=P)
        for r in range(NCHUNK):
            nc.sync.dma_start(out=E_sb[:, r, :], in_=emb_r[:, r, :])
        for r in range(NCHUNK):
            nc.sync.dma_start(out=A_sb[:, r, :], in_=anch_r[:, r, :])
        nc.sync.dma_start(out=Q_sb[:], in_=cov.rearrange("(b p) d -> p b d", p=P))
        lab_i = cst.tile([P, NCHUNK], I32)
        neg_i = cst.tile([P, NCHUNK], I32)
        with nc.allow_non_contiguous_dma(reason="16KB one-time label load"):
            nc.sync.dma_start(out=lab_i[:], in_=labels.rearrange("(c p) -> p c", p=P))
            nc.sync.dma_start(out=neg_i[:], in_=neg.rearrange("(c p) -> p c", p=P))

        # ---- constants ----
        lab_f = cst.tile([P, NCHUNK], F32)
        neg_f = cst.tile([P, NCHUNK], F32)
        nc.vector.tensor_copy(lab_f[:], lab_i[:])
        nc.vector.tensor_copy(neg_f[:], neg_i[:])
        iota_row_i = cst.tile([P, P], I32)
        nc.gpsimd.iota(iota_row_i[:], pattern=[[1, P]], base=0, channel_multiplier=0)
        iota_row = cst.tile([P, P], F32)
        nc.vector.tensor_copy(iota_row[:], iota_row_i[:])
        iota_part_i = cst.tile([P, P], I32)
        nc.gpsimd.iota(iota_part_i[:], pattern=[[0, P]], base=0, channel_multiplier=1)
        iota_part = cst.tile([P, P], F32)
        nc.vector.tensor_copy(iota_part[:], iota_part_i[:])
        ident = cst.tile([P, P], F32)
        make_identity(nc, ident[:])
        negI = cst.tile([P, P], F32)
        nc.vector.tensor_scalar_mul(negI[:], ident[:], -1.0)
        ones_col = cst.tile([P, 1], F32)
        nc.vector.memset(ones_col[:], 1.0)
        eps_col = cst.tile([P, 1], F32)
        nc.vector.memset(eps_col[:], EPS)

        # ---- phase 1: local class sums + counts ----
        cc_sb = big.tile([P, D + 1], F32)
        with tc.tile_pool(name="p1", bufs=1, space="PSUM") as p1:
            ps_sums = p1.tile([P, D], F32)
            ps_cnt = p1.tile([P, 1], F32)
            for r in range(NCHUNK):
                oh = wp.tile([P, P], F32, tag="oh")
                nc.vector.tensor_tensor(
                    out=oh[:], in0=lab_f[:, r:r + 1].to_broadcast([P, P]),
                    in1=iota_row[:], op=OP.is_equal)
                nc.tensor.matmul(out=ps_sums[:], lhsT=oh[:],
                                 rhs=E_sb[:, r, :],
                                 start=(r == 0), stop=(r == NCHUNK - 1))
                nc.tensor.matmul(out=ps_cnt[:], lhsT=oh[:], rhs=ones_col[:],
                                 start=(r == 0), stop=(r == NCHUNK - 1))
            nc.vector.tensor_copy(cc_sb[:, :D], ps_sums[:])
            nc.vector.tensor_copy(cc_sb[:, D:D + 1], ps_cnt[:])

        # ---- phase 2: AllReduce class sums/counts across the 8 cores ----
        cc_in = dram.tile([P, D + 1], F32)
        cc_out = dram.tile([P, D + 1], F32)
        nc.gpsimd.dma_start(out=cc_in[:], in_=cc_sb[:])
        nc.gpsimd.collective_compute(
            "AllReduce", OP.add, replica_groups=[list(range(N_CORES))],
            ins=[cc_in.opt()], outs=[cc_out.opt()])
        S_sb = big.tile([P, D + 1], F32)
        nc.gpsimd.dma_start(out=S_sb[:], in_=cc_out[:])

        S_r = big.tile([P, D], F32R)
        nc.vector.tensor_copy(S_r[:], S_sb[:, :D])
        Q_r = big.tile([P, DB, D], F32R)
        nc.vector.tensor_copy(Q_r[:], Q_sb[:])

        # per-class reciprocals: rp = 1/max(cnt-1,1), rn = 1/max(cnt,1)
        rp_col = cst.tile([P, 1], F32)
        rn_col = cst.tile([P, 1], F32)
        t_col = cst.tile([P, 1], F32)
        nc.vector.tensor_scalar(t_col[:], S_sb[:, D:D + 1], -1.0, 1.0,
                                OP.add, OP.max)
        nc.vector.reciprocal(rp_col[:], t_col[:])
        nc.vector.tensor_scalar(t_col[:], S_sb[:, D:D + 1], 1.0, None, OP.max)
        nc.vector.reciprocal(rn_col[:], t_col[:])

        # ---- phase 3: per-chunk gather, diff, transpose, matmul, dot ----
        d2p = big.tile([P, NCHUNK], F32)
        d2n = big.tile([P, NCHUNK], F32)
        pg = ctx.enter_context(tc.tile_pool(name="pg", bufs=2, space="PSUM"))
        psc = ctx.enter_context(tc.tile_pool(name="psc", bufs=2, space="PSUM"))
        pt = ctx.enter_context(tc.tile_pool(name="pt", bufs=2, space="PSUM"))
        pL = ctx.enter_context(tc.tile_pool(name="pL", bufs=2, space="PSUM"))
        for r in range(NCHUNK):
            # one-hot (classes on partitions) for pos and neg labels
            labT = pt.tile([P, P], F32, tag="tr")
            nc.tensor.transpose(labT[:], lab_f[:, r:r + 1].to_broadcast([P, P]),
                                ident[:])
            Bp = wp.tile([P, P], F32R, tag="Bp")
            nc.vector.tensor_tensor(out=Bp[:], in0=labT[:], in1=iota_part[:],
                                    op=OP.is_equal)
            negT = pt.tile([P, P], F32, tag="tr")
            nc.tensor.transpose(negT[:], neg_f[:, r:r + 1].to_broadcast([P, P]),
                                ident[:])
            Bn = wp.tile([P, P], F32R, tag="Bn")
            nc.vector.tensor_tensor(out=Bn[:], in0=negT[:], in1=iota_part[:],
                                    op=OP.is_equal)

            # per-sample reciprocals via exact one-hot gather (fp32 matmul)
            sc = psc.tile([P, 2], F32, tag="sc")
            nc.tensor.matmul(out=sc[:, 0:1], lhsT=Bp[:].bitcast(F32),
                             rhs=rp_col[:], start=True, stop=True)
            nc.tensor.matmul(out=sc[:, 1:2], lhsT=Bn[:].bitcast(F32),
                             rhs=rn_col[:], start=True, stop=True)

            # mp = gather(sums)[labels] - E ; mn = gather(sums)[neg]
            mp = pg.tile([P, D], F32, tag="g")
            nc.tensor.matmul(out=mp[:], lhsT=Bp[:], rhs=S_r[:],
                             start=True, stop=False)
            nc.tensor.matmul(out=mp[:], lhsT=negI[:], rhs=E_sb[:, r, :],
                             start=False, stop=True)
            dfp = wp.tile([P, D], F32, tag="dfp")
            nc.vector.scalar_tensor_tensor(
                out=dfp[:], in0=mp[:], scalar=sc[:, 0:1], in1=A_sb[:, r, :],
                op0=OP.mult, op1=OP.subtract)
            mn = pg.tile([P, D], F32, tag="g")
            nc.tensor.matmul(out=mn[:], lhsT=Bn[:], rhs=S_r[:],
                             start=True, stop=True)
            dfn = wp.tile([P, D], F32, tag="dfn")
            nc.vector.scalar_tensor_tensor(
                out=dfn[:], in0=mn[:], scalar=sc[:, 1:2], in1=A_sb[:, r, :],
                op0=OP.mult, op1=OP.subtract)

            # transpose diffs (PE) for the contraction over d
            dfpT = wp.tile([P, DB, P], F32R, tag="dfpT")
            dfnT = wp.tile([P, DB, P], F32R, tag="dfnT")
            for b in range(DB):
                tp = pt.tile([P, P], F32, tag="tr")
                nc.tensor.transpose(tp[:], dfp[:, b * P:(b + 1) * P], ident[:])
                nc.any.tensor_copy(out=dfpT[:, b, :], in_=tp[:])
                tn = pt.tile([P, P], F32, tag="tr")
                nc.tensor.transpose(tn[:], dfn[:, b * P:(b + 1) * P], ident[:])
                nc.any.tensor_copy(out=dfnT[:, b, :], in_=tn[:])

            # L = diff @ cov_inv  (fp32r, full rate), then d2 = rowsum(L*diff)
            Lp = pL.tile([P, D], F32, tag="L")
            for b in range(DB):
                nc.tensor.matmul(out=Lp[:], lhsT=dfpT[:, b, :],
                                 rhs=Q_r[:, b, :],
                                 start=(b == 0), stop=(b == DB - 1))
            sc1 = wp.tile([P, D], mybir.dt.bfloat16, tag="scr")
            nc.vector.scalar_tensor_tensor(
                out=sc1[:], in0=Lp[:], scalar=1.0, in1=dfp[:],
                op0=OP.mult, op1=OP.mult, accum_out=d2p[:, r:r + 1])
            Ln = pL.tile([P, D], F32, tag="L")
            for b in range(DB):
                nc.tensor.matmul(out=Ln[:], lhsT=dfnT[:, b, :],
                                 rhs=Q_r[:, b, :],
                                 start=(b == 0), stop=(b == DB - 1))
            sc2 = wp.tile([P, D], mybir.dt.bfloat16, tag="scr")
            nc.vector.scalar_tensor_tensor(
                out=sc2[:], in0=Ln[:], scalar=1.0, in1=dfn[:],
                op0=OP.mult, op1=OP.mult, accum_out=d2n[:, r:r + 1])

        # ---- epilogue: hinge + reductions ----
        posd = big.tile([P, NCHUNK], F32)
        negd = big.tile([P, NCHUNK], F32)
        nc.vector.tensor_scalar_max(posd[:], d2p[:], 0.0)
        nc.scalar.activation(posd[:], posd[:], AF.Sqrt, bias=eps_col[:])
        nc.vector.tensor_scalar_max(negd[:], d2n[:], 0.0)
        nc.scalar.activation(negd[:], negd[:], AF.Sqrt, bias=eps_col[:])
        h = big.tile([P, NCHUNK], F32)
        nc.vector.scalar_tensor_tensor(
            out=h[:], in0=posd[:], scalar=MARGIN, in1=negd[:],
            op0=OP.add, op1=OP.subtract)
        nc.vector.tensor_scalar_max(h[:], h[:], 0.0)
        red = cst.tile([P, 1], F32)
        nc.vector.tensor_reduce(red[:], h[:], axis=mybir.AxisListType.X,
                                op=OP.add)
        ptot = pL.tile([1, 1], F32, tag="L")
        nc.tensor.matmul(out=ptot[:], lhsT=red[:], rhs=ones_col[:],
                         start=True, stop=True)
        tot_sb = cst.tile([1, 1], F32)
        nc.vector.tensor_copy(tot_sb[:], ptot[:])
        nc.sync.dma_start(out=out, in_=tot_sb[:])
    _split_multi_waits(nc)
    return nc


_NC_CACHE = None


def kernel(**inputs):
    global _NC_CACHE
    if _NC_CACHE is None:
        _NC_CACHE = build()
    nc = _NC_CACHE
    anchor = np.ascontiguousarray(inputs["anchor"], dtype=np.float32)
    emb = np.ascontiguousarray(inputs["embeddings"], dtype=np.float32)
    cov = np.ascontiguousarray(inputs["cov_inv"], dtype=np.float32)
    labels = np.ascontiguousarray(inputs["labels"], dtype=np.int32)
    neg = np.ascontiguousarray(inputs["neg_labels"], dtype=np.int32)
    in_maps = []
    for i in range(N_CORES):
        sl = slice(i * R, (i + 1) * R)
        in_maps.append({
            "anchor": anchor[sl], "embeddings": emb[sl], "cov_inv": cov,
            "labels": labels[sl], "neg_labels": neg[sl],
        })
    res = run_bass_kernel_spmd(nc, in_maps, list(range(N_CORES)))
    total = sum(float(res.results[i]["partial"][0, 0]) for i in range(N_CORES))
    return np.float32(total / B)


# revision 10
# speedup vs baseline: 1.6998x; 1.6998x over previous
"""CentroidTripletLoss on 8 Trainium2 NeuronCores (Bass/Tile).

Data-parallel over batch B=32768 across 8 cores (4096 rows each), cov_inv
replicated. Per core: one-hot matmul segment-sums (bf16) -> AllReduce
(C*D + C), overlapped with building all per-chunk one-hot gather matrices ->
per-chunk one-hot gather matmuls (fp32r) with the embedding fold done on the
PE in bf16 -> fused DVE diff build -> PE transposes -> fp32r matmuls
diff @ cov_inv -> fused multiply + free-axis-accumulate for the Mahalanobis
quadratic form -> hinge epilogue. Each core emits its partial hinge sum; the
host sums and divides by B.
"""
import sys

sys.path.insert(0, "/opt/trn_rl_repo")

import numpy as np

import concourse.bass as bass
import concourse.mybir as mybir
import concourse.tile as tile
from concourse.bass_utils import run_bass_kernel_spmd
from concourse.masks import make_identity

N_CORES = 8
B, D, C = 32768, 512, 128
R = B // N_CORES            # rows per core (4096)
P = 128                     # partitions / chunk rows
NCHUNK = R // P             # 32 chunks per core
DB = D // P                 # 4 d-blocks
MARGIN = 0.1
EPS = 1e-12

F32 = mybir.dt.float32
F32R = mybir.dt.float32r
BF16 = mybir.dt.bfloat16
I32 = mybir.dt.int32
AF = mybir.ActivationFunctionType
OP = mybir.AluOpType


def _patch_drain_and_barrier():
    """The walrus build in this container rejects >1 sync wait per
    instruction; Tile's kernel-tail drain piles every outstanding semaphore
    wait onto one SP Drain. Split it into a chain of single-wait drains."""
    import bass_rust as _br
    from concourse.vector_clock import ScopedClock

    def _patched(self, tick_clock, wait_clock):
        nc = self.nc
        drain_inst = nc.sync.drain()
        wait_clock.add_sem_waits(
            drain_inst.ins, ScopedClock({None: tick_clock.global_clock}))
        waits = list(drain_inst.ins.sync_info.on_wait)
        if len(waits) > 1:
            drain_inst.ins.sync_info.on_wait = waits[:1]
            for w in waits[1:]:
                d = nc.sync.drain()
                if d.ins.sync_info is None:
                    d.ins.sync_info = _br.SyncInfo(on_wait=[w], on_update=[])
                else:
                    d.ins.sync_info.on_wait = [w]
        nc.all_engine_barrier()
        popped = nc._tile_sem_poison_stack.pop()
        assert popped is self._sem_poison
        nc.clear_and_free_semaphores(list(self.sems.allocated().values()))
        nc.all_engine_barrier()

    tile.TileContext._drain_and_barrier = _patched


_patch_drain_and_barrier()


def _split_multi_waits(nc):
    """The walrus build here accepts 1 sync wait per instruction (2 for
    EventSemaphore). Move extra waits onto same-engine NoOps inserted just
    before the over-subscribed instruction."""
    import bass_rust as _br
    k = 0
    for fn in nc.m.functions:
        for bb in fn.blocks:
            insts = bb.instructions
            out = []
            changed = False
            for ins in insts:
                si = ins.sync_info
                cap = 2 if isinstance(ins, mybir.InstEventSemaphore) else 1
                if si is not None and len(si.on_wait) > cap:
                    waits = list(si.on_wait)
                    for w in waits[:-cap]:
                        nop = mybir.InstNoOp(name=f"swsplit_{k}", ins=[], outs=[])
                        k += 1
                        nop.engine = ins.engine
                        nop.sync_info = _br.SyncInfo(on_wait=[w], on_update=[])
                        out.append(nop)
                    si.on_wait = waits[-cap:]
                    changed = True
                out.append(ins)
            if changed:
                bb.instructions = out


def build():
    nc = bass.Bass("TRN2", target_bir_lowering=False, debug=False,
                   num_devices=N_CORES)
    anchor = nc.dram_tensor("anchor", [R, D], F32, kind="ExternalInput").ap()
    emb = nc.dram_tensor("embeddings", [R, D], F32, kind="ExternalInput").ap()
    cov = nc.dram_tensor("cov_inv", [D, D], F32, kind="ExternalInput").ap()
    labels = nc.dram_tensor("labels", [R], I32, kind="ExternalInput").ap()
    neg = nc.dram_tensor("neg_labels", [R], I32, kind="ExternalInput").ap()
    out = nc.dram_tensor("partial", [1, 1], F32, kind="ExternalOutput").ap()

    from contextlib import ExitStack
    with tile.TileContext(nc) as tc, ExitStack() as ctx:
        cst = ctx.enter_context(tc.tile_pool(name="cst", bufs=1))
        big = ctx.enter_context(tc.tile_pool(name="big", bufs=1))
        wp = ctx.enter_context(tc.tile_pool(name="wp", bufs=3))
        ap_pool = ctx.enter_context(tc.tile_pool(name="ap", bufs=4))
        dram = ctx.enter_context(tc.tile_pool(name="dram", bufs=1, space="DRAM"))
        pt = ctx.enter_context(tc.tile_pool(name="pt", bufs=2, space="PSUM"))

        emb_r = emb.rearrange("(c p) d -> p c d", p=P)
        anch_r = anchor.rearrange("(c p) d -> p c d", p=P)

        # ---- constants ----
        iota_row_i = cst.tile([P, P], I32)
        nc.gpsimd.iota(iota_row_i[:], pattern=[[1, P]], base=0, channel_multiplier=0)
        iota_row = cst.tile([P, P], F32)
        nc.vector.tensor_copy(iota_row[:], iota_row_i[:])
        iota_part_i = cst.tile([P, P], I32)
        nc.gpsimd.iota(iota_part_i[:], pattern=[[0, P]], base=0, channel_multiplier=1)
        iota_part = cst.tile([P, P], F32)
        nc.vector.tensor_copy(iota_part[:], iota_part_i[:])
        ident = cst.tile([P, P], F32)
        make_identity(nc, ident[:])
        negI_bf = cst.tile([P, P], BF16)
        nc.vector.tensor_scalar_mul(negI_bf[:], ident[:], -1.0)
        ones_col = cst.tile([P, 1], BF16)
        nc.vector.memset(ones_col[:], 1.0)
        ones_f32 = cst.tile([P, 1], F32)
        nc.vector.memset(ones_f32[:], 1.0)
        eps_col = cst.tile([P, 1], F32)
        nc.vector.memset(eps_col[:], EPS)

        # labels: natural [NCHUNK, P] layout (contiguous DMA), then one PE
        # transpose to [P, NCHUNK] so chunk r's labels sit on partitions.
        lab_nat = cst.tile([NCHUNK, P], I32)
        neg_nat = cst.tile([NCHUNK, P], I32)
        nc.sync.dma_start(out=lab_nat[:], in_=labels.rearrange("(c p) -> c p", p=P))
        nc.sync.dma_start(out=neg_nat[:], in_=neg.rearrange("(c p) -> c p", p=P))
        lab_natf = cst.tile([NCHUNK, P], F32)
        neg_natf = cst.tile([NCHUNK, P], F32)
        nc.vector.tensor_copy(lab_natf[:], lab_nat[:])
        nc.vector.tensor_copy(neg_natf[:], neg_nat[:])
        lab_f = cst.tile([P, NCHUNK], F32)
        neg_f = cst.tile([P, NCHUNK], F32)
        tl = pt.tile([P, NCHUNK], F32, tag="tr")
        nc.tensor.transpose(tl[:], lab_natf[:], ident[:NCHUNK, :NCHUNK])
        nc.vector.tensor_copy(lab_f[:], tl[:])
        tl2 = pt.tile([P, NCHUNK], F32, tag="tr")
        nc.tensor.transpose(tl2[:], neg_natf[:], ident[:NCHUNK, :NCHUNK])
        nc.vector.tensor_copy(neg_f[:], tl2[:])

        # ---- phase 1: stream E, cast to bf16, one-hot segment sums ----
        E_bf = big.tile([P, NCHUNK, D], BF16)
        cc_sb = big.tile([P, D + 1], F32)
        with tc.tile_pool(name="p1", bufs=1, space="PSUM") as p1:
            ps_sums = p1.tile([P, D], F32)
            ps_cnt = p1.tile([P, 1], F32)
            for r in range(NCHUNK):
                e_chunk = ap_pool.tile([P, D], F32, tag="ec")
                nc.sync.dma_start(out=e_chunk[:], in_=emb_r[:, r, :])
                nc.vector.tensor_copy(E_bf[:, r, :], e_chunk[:])
                oh = wp.tile([P, P], BF16, tag="oh")
                nc.vector.tensor_tensor(
                    out=oh[:], in0=lab_f[:, r:r + 1].to_broadcast([P, P]),
                    in1=iota_row[:], op=OP.is_equal)
                nc.tensor.matmul(out=ps_sums[:], lhsT=oh[:],
                                 rhs=E_bf[:, r, :],
                                 start=(r == 0), stop=(r == NCHUNK - 1))
                nc.tensor.matmul(out=ps_cnt[:], lhsT=oh[:], rhs=ones_col[:],
                                 start=(r == 0), stop=(r == NCHUNK - 1))
            nc.vector.tensor_copy(cc_sb[:, :D], ps_sums[:])
            nc.vector.tensor_copy(cc_sb[:, D:D + 1], ps_cnt[:])

        # ---- one-hot gather matrices for every chunk (AllReduce-independent,
        # emitted before the collective so they fill the AllReduce stall) ----
        Bp_all = big.tile([P, NCHUNK, P], F32R)
        Bn_all = big.tile([P, NCHUNK, P], F32R)
        for r in range(NCHUNK):
            labT = pt.tile([P, P], F32, tag="tr")
            nc.tensor.transpose(labT[:], lab_f[:, r:r + 1].to_broadcast([P, P]),
                                ident[:])
            nc.vector.tensor_tensor(out=Bp_all[:, r, :], in0=labT[:],
                                    in1=iota_part[:], op=OP.is_equal)
            negT = pt.tile([P, P], F32, tag="tr")
            nc.tensor.transpose(negT[:], neg_f[:, r:r + 1].to_broadcast([P, P]),
                                ident[:])
            nc.vector.tensor_tensor(out=Bn_all[:, r, :], in0=negT[:],
                                    in1=iota_part[:], op=OP.is_equal)

        # ---- cov_inv load + fp32r cast (also AllReduce-independent) ----
        Q_r = big.tile([P, DB, D], F32R)
        cov_r = cov.rearrange("(b p) d -> p b d", p=P)
        for b in range(DB):
            q_chunk = ap_pool.tile([P, D], F32, tag="qc")
            nc.sync.dma_start(out=q_chunk[:], in_=cov_r[:, b, :])
            nc.vector.tensor_copy(Q_r[:, b, :], q_chunk[:])

        # ---- phase 2: AllReduce class sums/counts across the 8 cores ----
        cc_in = dram.tile([P, D + 1], F32)
        cc_out = dram.tile([P, D + 1], F32)
        nc.gpsimd.dma_start(out=cc_in[:], in_=cc_sb[:])
        nc.gpsimd.collective_compute(
            "AllReduce", OP.add, replica_groups=[list(range(N_CORES))],
            ins=[cc_in.opt()], outs=[cc_out.opt()])
        S_sb = big.tile([P, D + 1], F32)
        nc.gpsimd.dma_start(out=S_sb[:], in_=cc_out[:])
        S_r = big.tile([P, D], F32R)
        nc.vector.tensor_copy(S_r[:], S_sb[:, :D])

        # per-class reciprocals: rp = 1/max(cnt-1,1), rn = 1/max(cnt,1)
        rp_col = cst.tile([P, 1], F32)
        rn_col = cst.tile([P, 1], F32)
        t_col = cst.tile([P, 1], F32)
        nc.vector.tensor_scalar(t_col[:], S_sb[:, D:D + 1], -1.0, 1.0,
                                OP.add, OP.max)
        nc.vector.reciprocal(rp_col[:], t_col[:])
        nc.vector.tensor_scalar(t_col[:], S_sb[:, D:D + 1], 1.0, None, OP.max)
        nc.vector.reciprocal(rn_col[:], t_col[:])

        # ---- phase 3: per-chunk gather, diff, transpose, matmul, dot ----
        d2p = big.tile([P, NCHUNK], F32)
        d2n = big.tile([P, NCHUNK], F32)
        pg = ctx.enter_context(tc.tile_pool(name="pg", bufs=2, space="PSUM"))
        psc = ctx.enter_context(tc.tile_pool(name="psc", bufs=2, space="PSUM"))
        pL = ctx.enter_context(tc.tile_pool(name="pL", bufs=2, space="PSUM"))
        for r in range(NCHUNK):
            a_chunk = ap_pool.tile([P, D], F32, tag="ac")
            nc.sync.dma_start(out=a_chunk[:], in_=anch_r[:, r, :])

            # per-sample reciprocals via exact one-hot gather (fp32 matmul)
            sc = psc.tile([P, 2], F32, tag="sc")
            nc.tensor.matmul(out=sc[:, 0:1], lhsT=Bp_all[:, r, :].bitcast(F32),
                             rhs=rp_col[:], start=True, stop=True)
            nc.tensor.matmul(out=sc[:, 1:2], lhsT=Bn_all[:, r, :].bitcast(F32),
                             rhs=rn_col[:], start=True, stop=True)

            # mp = gather(sums)[labels] - E ; mn = gather(sums)[neg]
            mp = pg.tile([P, D], F32, tag="g")
            nc.tensor.matmul(out=mp[:], lhsT=Bp_all[:, r, :], rhs=S_r[:],
                             start=True, stop=False)
            nc.tensor.matmul(out=mp[:], lhsT=negI_bf[:], rhs=E_bf[:, r, :],
                             start=False, stop=True)
            dfp = wp.tile([P, D], F32, tag="dfp")
            nc.vector.scalar_tensor_tensor(
                out=dfp[:], in0=mp[:], scalar=sc[:, 0:1], in1=a_chunk[:],
                op0=OP.mult, op1=OP.subtract)
            mn = pg.tile([P, D], F32, tag="g")
            nc.tensor.matmul(out=mn[:], lhsT=Bn_all[:, r, :], rhs=S_r[:],
                             start=True, stop=True)
            dfn = wp.tile([P, D], F32, tag="dfn")
            nc.vector.scalar_tensor_tensor(
                out=dfn[:], in0=mn[:], scalar=sc[:, 1:2], in1=a_chunk[:],
                op0=OP.mult, op1=OP.subtract)

            # transpose diffs (PE) for the contraction over d;
            # psum->sbuf copies split across Scalar (pos) and Vector (neg)
            dfpT = wp.tile([P, DB, P], F32R, tag="dfpT")
            dfnT = wp.tile([P, DB, P], F32R, tag="dfnT")
            for b in range(DB):
                tp = pt.tile([P, P], F32, tag="tr")
                nc.tensor.transpose(tp[:], dfp[:, b * P:(b + 1) * P], ident[:])
                nc.scalar.activation(dfpT[:, b, :], tp[:], AF.Copy)
                tn = pt.tile([P, P], F32, tag="tr")
                nc.tensor.transpose(tn[:], dfn[:, b * P:(b + 1) * P], ident[:])
                nc.vector.tensor_copy(dfnT[:, b, :], tn[:])

            # L = diff @ cov_inv  (fp32r, full rate), then d2 = rowsum(L*diff)
            Lp = pL.tile([P, D], F32, tag="L")
            for b in range(DB):
                nc.tensor.matmul(out=Lp[:], lhsT=dfpT[:, b, :],
                                 rhs=Q_r[:, b, :],
                                 start=(b == 0), stop=(b == DB - 1))
            sc1 = wp.tile([P, D], BF16, tag="scr")
            nc.vector.scalar_tensor_tensor(
                out=sc1[:], in0=Lp[:], scalar=1.0, in1=dfp[:],
                op0=OP.mult, op1=OP.mult, accum_out=d2p[:, r:r + 1])
            Ln = pL.tile([P, D], F32, tag="L")
            for b in range(DB):
                nc.tensor.matmul(out=Ln[:], lhsT=dfnT[:, b, :],
                                 rhs=Q_r[:, b, :],
                                 start=(b == 0), stop=(b == DB - 1))
            sc2 = wp.tile([P, D], BF16, tag="scr")
            nc.vector.scalar_tensor_tensor(
                out=sc2[:], in0=Ln[:], scalar=1.0, in1=dfn[:],
                op0=OP.mult, op1=OP.mult, accum_out=d2n[:, r:r + 1])

        # ---- epilogue: hinge + reductions ----
        posd = big.tile([P, NCHUNK], F32)
        negd = big.tile([P, NCHUNK], F32)
        nc.vector.tensor_scalar_max(posd[:], d2p[:], 0.0)
        nc.scalar.activation(posd[:], posd[:], AF.Sqrt, bias=eps_col[:])
        nc.vector.tensor_scalar_max(negd[:], d2n[:], 0.0)
        nc.scalar.activation(negd[:], negd[:], AF.Sqrt, bias=eps_col[:])
        h = big.tile([P, NCHUNK], F32)
        nc.vector.scalar_tensor_tensor(
            out=h[:], in0=posd[:], scalar=MARGIN, in1=negd[:],
            op0=OP.add, op1=OP.subtract)
        nc.vector.tensor_scalar_max(h[:], h[:], 0.0)
        red = cst.tile([P, 1], F32)
        nc.vector.tensor_reduce(red[:], h[:], axis=mybir.AxisListType.X,
                                op=OP.add)
        ptot = pL.tile([1, 1], F32, tag="L")
        nc.tensor.matmul(out=ptot[:], lhsT=red[:], rhs=ones_f32[:],
                         start=True, stop=True)
        tot_sb = cst.tile([1, 1], F32)
        nc.vector.tensor_copy(tot_sb[:], ptot[:])
        nc.sync.dma_start(out=out, in_=tot_sb[:])
    _split_multi_waits(nc)
    return nc


_NC_CACHE = None


def kernel(**inputs):
    global _NC_CACHE
    if _NC_CACHE is None:
        _NC_CACHE = build()
    nc = _NC_CACHE
    anchor = np.ascontiguousarray(inputs["anchor"], dtype=np.float32)
    emb = np.ascontiguousarray(inputs["embeddings"], dtype=np.float32)
    cov = np.ascontiguousarray(inputs["cov_inv"], dtype=np.float32)
    labels = np.ascontiguousarray(inputs["labels"], dtype=np.int32)
    neg = np.ascontiguousarray(inputs["neg_labels"], dtype=np.int32)
    in_maps = []
    for i in range(N_CORES):
        sl = slice(i * R, (i + 1) * R)
        in_maps.append({
            "anchor": anchor[sl], "embeddings": emb[sl], "cov_inv": cov,
            "labels": labels[sl], "neg_labels": neg[sl],
        })
    res = run_bass_kernel_spmd(nc, in_maps, list(range(N_CORES)))
    total = sum(float(res.results[i]["partial"][0, 0]) for i in range(N_CORES))
    return np.float32(total / B)
